# revision 24
# baseline (speedup 1.0000x reference)
"""nn_Attention_18700287607351 — GQA attention (RMSNorm + RoPE + causal) on
8 TRN2 NeuronCores via Bass/Tile.

Sharding (hardcoded): 8 shards = (batch b in {0,1}) x (4 KV-head groups per
batch). Each shard owns 2 KV heads and their 8 query heads (GQA repeat stays
local); Wq/Wk/Wv rows and Wo columns are split by head group. Per-shard
partial Wo products are summed across the 4 groups of a batch on the host at
gather time (row-parallel output projection).

Device-side layout notes:
- q/k-head and attention-output transposes run as SBUF->SBUF DMA xbar
  transposes (idle DMA engines); softmax-probability transposes stay on
  TensorE (PE transpose + PSUM->SBUF copy) — measured faster than the DMA
  path for that volume.
- Query heads are reordered host-side so position j holds head (j//2)+4*(j%2);
  head pairs (h, h+4) then share a 128-partition transposed tile whose halves
  line up with the packed [kv0; kv1] kT tile (GQA stays a partition offset).
- RoPE tables fold in the per-head norm weight (and softmax scale for q).
- Emission order is software-pipelined: Wo for tile i-1 is emitted at tile i
  (oT always ready), and P-transpose/PV for head j after scores for head j+1
  (exp latency never blocks TensorE's in-order stream).

Shapes (must match the grader's setup_inputs):
  x  [2, 2048, 2048]  Wq [2048, 2048]  Wk [512, 2048]  Wv [512, 2048]
  Wo [2048, 2048]     q_norm_w [64]    k_norm_w [64]   mask [1,1,2048,2048]
"""

import sys
import types
import numpy as np
import ml_dtypes

BF16 = ml_dtypes.bfloat16

B, T, D = 2, 2048, 2048
H, HKV, HD = 32, 8, 64
THETA = 3.0
EPS = 1e-6
SCALE = HD ** -0.5
N_CORES = 8
QH = 8            # query heads per shard
KVH = 2           # kv heads per shard
NT = T // 128     # 16 t-tiles
ND = D // 128     # 16 d-tiles
MQ = QH * HD      # 512 q rows per shard
MKV = KVH * HD    # 128 kv rows per shard
# position j in the permuted q block holds head (j//2) + 4*(j%2)
PERM = [(j // 2) + 4 * (j % 2) for j in range(QH)]


# ---------------------------------------------------------------------------
# Workaround: walrus in this env rejects >1 sem wait on one instruction.
# The kernel-tail Tile drain collects one wait per outstanding proc — split
# those across preceding SP NOPs.
def _install_tile_drain_patch():
    import bass_rust
    import concourse.tile as tile
    import concourse.mybir as mybir

    if getattr(tile.TileContext, "_drain_split_installed", False):
        return

    def _patched(self, tick_clock, wait_clock):
        nops = [self.nc.sync.nop(nofuse=True, hint=f"drain_split_{i}").ins
                for i in range(64)]
        drain_inst = self.nc.sync.drain()
        wait_clock.add_sem_waits(
            drain_inst.ins, bass_rust.ScopedClock({None: tick_clock.global_clock})
        )
        waits = list(drain_inst.ins.sync_info.on_wait or [])
        if len(waits) > 1:
            drain_inst.ins.sync_info.on_wait = waits[:1]
            rest = waits[1:]
            assert len(rest) <= len(nops), f"too many drain waits: {len(waits)}"
            for nop, w in zip(nops, rest):
                si = nop.sync_info
                if si is None:
                    nop.sync_info = mybir.SyncInfo(on_wait=[w], on_update=[])
                else:
                    si.on_wait = [w]

        self.nc.all_engine_barrier()
        assert self.sems is not None
        popped = self.nc._tile_sem_poison_stack.pop()
        assert popped is self._sem_poison
        self.nc.clear_and_free_semaphores(list(self.sems.allocated().values()))
        self.nc.all_engine_barrier()

    tile.TileContext._drain_and_barrier = _patched
    tile.TileContext._drain_split_installed = True


# Same walrus limit, general form: split any instruction's waits beyond the
# first onto single-wait EventSemaphore instructions inserted just before it
# (same engine, same basic block) at the BIR-JSON level.
def _install_wait_split_patch():
    import concourse.bass2jax as b2j

    if getattr(b2j, "_wait_split_installed", False):
        return
    orig = b2j.compile_bir_kernel

    def _split_waits(bir_json):
        import orjson
        d = orjson.loads(bir_json)
        ctr = 0
        changed = False
        for fn in d.get("functions", []):
            for bb in fn.get("blocks", []):
                insts = bb.get("instructions", [])
                out = []
                for inst in insts:
                    si = inst.get("sync_info")
                    waits = (si or {}).get("on_wait") or []
                    cap = 1
                    if len(waits) > cap:
                        changed = True
                        for w in waits[cap:]:
                            ctr += 1
                            out.append({
                                "engine": inst["engine"], "ins": [],
                                "name": f"wsplit{ctr}", "opcode": "EventSemaphore",
                                "outs": [],
                                "sync_info": {"on_update": [], "on_wait": [w]},
                            })
                        si["on_wait"] = waits[:cap]
                    out.append(inst)
                bb["instructions"] = out
        if not changed:
            return bir_json
        return orjson.dumps(d)

    def wrapped(bir_json, tmpdir, neff_name="file.neff"):
        return orig(_split_waits(bir_json), tmpdir, neff_name=neff_name)

    b2j.compile_bir_kernel = wrapped
    b2j._wait_split_installed = True


# If tracing gets requested (BASS_TRACE env or trace=True), concourse needs
# antenv.axon_hooks, which this image lacks. Provide it.
def _install_axon_hooks_shim():
    if "antenv.axon_hooks" in sys.modules:
        return
    mod = types.ModuleType("antenv.axon_hooks")
    mod._hook = None
    mod.set_axon_ntff_profile_hook = lambda h: setattr(mod, "_hook", h)
    mod.get_axon_ntff_profile_hook = lambda: mod._hook
    sys.modules["antenv.axon_hooks"] = mod
    import antenv
    antenv.axon_hooks = mod
    try:
        from trn_agent_boot.trn_boot import _ntff_profile_via_ctypes
        mod._hook = _ntff_profile_via_ctypes("/opt/axon/libaxon_pjrt.so")
    except Exception:
        pass
    try:
        import concourse.bass_utils as bu
        bu.upload_artifacts = lambda tmpdir: str(tmpdir)
    except Exception:
        pass


# ---------------------------------------------------------------------------
# Device program (SPMD — identical on all 8 cores; inputs differ per core).
def _build_nc():
    import contextlib
    import concourse.bass as bass
    import concourse.mybir as mybir
    import concourse.tile as tile

    _install_tile_drain_patch()

    f32 = mybir.dt.float32
    bf16 = mybir.dt.bfloat16
    AX = mybir.AxisListType.X
    EXP = mybir.ActivationFunctionType.Exp
    SQRT = mybir.ActivationFunctionType.Sqrt
    MULT = mybir.AluOpType.mult

    nc = bass.Bass()
    xT = nc.declare_dram_parameter("xT", [D, T], bf16, isOutput=False)
    wqkvT = nc.declare_dram_parameter("wqkvT", [D, MQ + 2 * MKV], bf16, isOutput=False)
    woT = nc.declare_dram_parameter("woT", [MQ, D], bf16, isOutput=False)
    ropeq = nc.declare_dram_parameter("ropeq", [128, NT * 128], f32, isOutput=False)
    ropek = nc.declare_dram_parameter("ropek", [128, NT * 128], f32, isOutput=False)
    madd = nc.declare_dram_parameter("madd", [128, NT * 128], f32, isOutput=False)
    ident = nc.declare_dram_parameter("ident", [128, 128], bf16, isOutput=False)
    yout = nc.declare_dram_parameter("out", [T, D], bf16, isOutput=True)

    with tile.TileContext(nc) as tc, contextlib.ExitStack() as ctx:
        const = ctx.enter_context(tc.tile_pool(name="const", bufs=1))
        persist = ctx.enter_context(tc.tile_pool(name="persist", bufs=1))
        proj = ctx.enter_context(tc.tile_pool(name="proj", bufs=1))
        work = ctx.enter_context(tc.tile_pool(name="work", bufs=2))
        stat = ctx.enter_context(tc.tile_pool(name="stat", bufs=2))
        ppool = ctx.enter_context(tc.tile_pool(name="P", bufs=4))
        psq = ctx.enter_context(tc.tile_pool(name="ps_qkv", bufs=1, space="PSUM"))
        pskv = ctx.enter_context(tc.tile_pool(name="ps_kv", bufs=1, space="PSUM"))
        pssc = ctx.enter_context(tc.tile_pool(name="ps_sc", bufs=2, space="PSUM"))
        pso = ctx.enter_context(tc.tile_pool(name="ps_o", bufs=1, space="PSUM"))
        psm = ctx.enter_context(tc.tile_pool(name="ps_misc", bufs=3, space="PSUM"))

        # ---- constants / weights ----
        id_sb = const.tile([128, 128], bf16, tag="ident")
        nc.sync.dma_start(id_sb[:], ident[:])
        rq_sb = const.tile([128, NT * 128], f32, tag="ropeq")
        nc.sync.dma_start(rq_sb[:], ropeq[:])
        rk_sb = const.tile([128, NT * 128], f32, tag="ropek")
        nc.sync.dma_start(rk_sb[:], ropek[:])
        md_sb = const.tile([128, NT * 128], f32, tag="madd")
        nc.sync.dma_start(md_sb[:], madd[:])
        epsb = const.tile([128, 1], f32, tag="epsb")
        nc.vector.memset(epsb[:], EPS)

        wo_sb = []
        for mt in range(4):
            t_ = const.tile([128, D], bf16, tag=f"woT{mt}")
            nc.sync.dma_start(t_[:], woT.rearrange("(a p) n -> a p n", p=128)[mt])
            wo_sb.append(t_)

        xT_sb = []
        wq_sb = []
        xT_r = xT.rearrange("(a p) t -> a p t", p=128)
        wq_r = wqkvT.rearrange("(a p) m -> a p m", p=128)
        for dt_ in range(ND):
            tx = proj.tile([128, T], bf16, tag=f"xT{dt_}")
            nc.sync.dma_start(tx[:], xT_r[dt_])
            xT_sb.append(tx)
            tw = proj.tile([128, MQ + 2 * MKV], bf16, tag=f"wqkvT{dt_}")
            nc.sync.dma_start(tw[:], wq_r[dt_])
            wq_sb.append(tw)

        # persistent per-shard tensors
        # kT packed: partitions 0:64 = kv head 0, 64:128 = kv head 1
        kT_sb = persist.tile([128, T], bf16, tag="kT", name="kT")
        v_sb = [persist.tile([128, MKV], bf16, tag=f"v{i}", name=f"v{i}")
                for i in range(NT)]
        oT_sb = [persist.tile([128, T], bf16, tag=f"oT{mt}", name=f"oT{mt}")
                 for mt in range(4)]

        def _emit_wo(iw):
            tw_ = slice(iw * 128, (iw + 1) * 128)
            for cc in range(4):
                y_ps = psm.tile([128, 512], f32, tag="pty", name="y_ps")
                for mt in range(4):
                    nc.tensor.matmul(y_ps[:], oT_sb[mt][:, tw_],
                                     wo_sb[mt][:, cc * 512:(cc + 1) * 512],
                                     start=(mt == 0), stop=(mt == 3))
                y_sb = work.tile([128, 512], bf16, tag="y_sb", name="y_sb")
                nc.vector.tensor_copy(y_sb[:], y_ps[:])
                nc.scalar.dma_start(yout[tw_, cc * 512:(cc + 1) * 512], y_sb[:])

        for i in range(NT):
            tsl = slice(i * 128, (i + 1) * 128)
            L = (i + 1) * 128  # causal row length for this tq-tile

            # ---- QKV projection for t-tile i ----
            q_ps = psq.tile([128, MQ], f32, tag="q")
            kv_ps = pskv.tile([128, 2 * MKV], f32, tag="kv")
            for dt_ in range(ND):
                st = xT_sb[dt_][:, tsl]
                nc.tensor.matmul(q_ps[:], st, wq_sb[dt_][:, 0:MQ],
                                 start=(dt_ == 0), stop=(dt_ == ND - 1))
                nc.tensor.matmul(kv_ps[:], st, wq_sb[dt_][:, MQ:MQ + 2 * MKV],
                                 start=(dt_ == 0), stop=(dt_ == ND - 1))

            # ---- RMS norm + RoPE (q: 8 heads, k: 2 heads) ----
            sq = work.tile([128, MQ], f32, tag="sq")
            nc.scalar.square(sq[:], q_ps[:])
            ms = stat.tile([128, QH + KVH], f32, tag="ms")
            nc.vector.reduce_sum(ms[:, 0:QH],
                                 sq[:].rearrange("p (h d) -> p h d", d=HD), axis=AX)
            sqk = work.tile([128, MKV], f32, tag="sqk")
            nc.scalar.square(sqk[:], kv_ps[:, 0:MKV])
            nc.vector.reduce_sum(ms[:, QH:QH + KVH],
                                 sqk[:].rearrange("p (h d) -> p h d", d=HD), axis=AX)
            srt = stat.tile([128, QH + KVH], f32, tag="srt")
            nc.scalar.activation(srt[:], ms[:], SQRT, bias=epsb[:], scale=1.0 / HD)
            rms = stat.tile([128, QH + KVH], f32, tag="rms")
            nc.vector.reciprocal(rms[:], srt[:])

            # qn = q * rms (broadcast over 64), same for k
            qn = work.tile([128, MQ], f32, tag="qn")
            nc.vector.tensor_tensor(
                qn[:].rearrange("p (h d) -> p h d", d=HD),
                q_ps[:].rearrange("p (h d) -> p h d", d=HD),
                rms[:, 0:QH].unsqueeze(-1).broadcast_to([128, QH, HD]),
                op=MULT)
            kn = work.tile([128, MKV], f32, tag="kn")
            nc.vector.tensor_tensor(
                kn[:].rearrange("p (h d) -> p h d", d=HD),
                kv_ps[:, 0:MKV].rearrange("p (h d) -> p h d", d=HD),
                rms[:, QH:QH + KVH].unsqueeze(-1).broadcast_to([128, KVH, HD]),
                op=MULT)

            # RoPE: out_e = qe*A - qo*B ; out_o = qe*C + qo*D
            # (tables have w and, for q, SCALE folded in)
            q_sb = work.tile([128, MQ], bf16, tag="q_sb")
            k_sb = work.tile([128, MKV], bf16, tag="k_sb")
            for (dst, src, tab, nh) in ((q_sb, qn, rq_sb, QH),
                                        (k_sb, kn, rk_sb, KVH)):
                sv = src[:].rearrange("p (h w two) -> p h w two", two=2, w=HD // 2)
                dv = dst[:].rearrange("p (h w two) -> p h w two", two=2, w=HD // 2)
                se, so = sv[:, :, :, 0], sv[:, :, :, 1]
                de, do = dv[:, :, :, 0], dv[:, :, :, 1]
                A = tab[:, i * 128 + 0:i * 128 + 32].unsqueeze(1).broadcast_to([128, nh, 32])
                Bt = tab[:, i * 128 + 32:i * 128 + 64].unsqueeze(1).broadcast_to([128, nh, 32])
                C = tab[:, i * 128 + 64:i * 128 + 96].unsqueeze(1).broadcast_to([128, nh, 32])
                Dt = tab[:, i * 128 + 96:i * 128 + 128].unsqueeze(1).broadcast_to([128, nh, 32])
                t1 = work.tile([128, nh * 32], f32, tag=f"rope_t1_{nh}", name="t1")
                t2 = work.tile([128, nh * 32], f32, tag=f"rope_t2_{nh}", name="t2")
                t1v = t1[:].rearrange("p (h w) -> p h w", w=32)
                t2v = t2[:].rearrange("p (h w) -> p h w", w=32)
                nc.vector.tensor_tensor(t1v, se, A, op=MULT)
                nc.vector.tensor_tensor(t2v, so, Bt, op=MULT)
                nc.vector.tensor_sub(de, t1v, t2v)
                nc.vector.tensor_tensor(t1v, se, C, op=MULT)
                nc.vector.tensor_tensor(t2v, so, Dt, op=MULT)
                nc.vector.tensor_add(do, t1v, t2v)

            # v -> bf16 sbuf
            nc.vector.tensor_copy(v_sb[i][:], kv_ps[:, MKV:2 * MKV])

            # ---- transposes via DMA xbar (SBUF->SBUF) ----
            # qT_i block p = transpose of q head-pair block p:
            #   partitions 0:64 = position 2p (kv0 head), 64:128 = position
            #   2p+1 (kv1 head); matches packed kT halves.
            qT_i = work.tile([128, 4 * 128], bf16, tag="qT")
            nc.sync.dma_start_transpose(
                qT_i[:].rearrange("p (c t) -> p c t", t=128), q_sb[:])
            nc.sync.dma_start_transpose(kT_sb[:, tsl], k_sb[:])

            # ---- output projection for t-tile i-1 (lagged so oT is ready) ----
            if i >= 1:
                _emit_wo(i - 1)

            # ---- attention for all 8 head positions, tq-tile i ----
            # P transpose + PV for head j are emitted AFTER scores for head
            # j+1, so TensorE never sits behind exp(j) in its in-order stream.
            o_ps = pso.tile([128, MQ], f32, tag="o")
            recip = stat.tile([128, QH], f32, tag="recip")
            nch = i // 4 + 1

            def _pv_phase(j, P):
                for gs in range(0, L, 512):
                    gw = min(512, L - gs)
                    gn = gw // 128
                    pt_ps = psm.tile([128, 512], bf16, tag="pty", name="pt_ps")
                    for jj in range(gn):
                        nc.tensor.transpose(
                            pt_ps[:, jj * 128:(jj + 1) * 128],
                            P[:, gs + jj * 128:gs + (jj + 1) * 128], id_sb[:])
                    pt_sb = work.tile([128, 512], bf16, tag="pt_sb", bufs=3)
                    nc.vector.tensor_copy(pt_sb[:, 0:gw], pt_ps[:, 0:gw])
                    for jj in range(gn):
                        jb = gs // 128 + jj
                        nc.tensor.matmul(
                            o_ps[:, j * HD:(j + 1) * HD],
                            pt_sb[:, jj * 128:(jj + 1) * 128],
                            v_sb[jb][:, (j % 2) * HD:(j % 2 + 1) * HD],
                            start=(jb == 0), stop=(jb == i))

            pend = []
            for j in range(QH):
                half = (j % 2) * 64
                pr = j // 2
                lhs_q = qT_i[half:half + 64, pr * 128:pr * 128 + 128]
                P = ppool.tile([128, NT * 128], bf16, tag="P", name="P")
                sums4 = stat.tile([128, 4], f32, tag="sums4")
                for c in range(nch):
                    wc = 512 if c < i // 4 else (i % 4 + 1) * 128
                    s_ps = pssc.tile([128, 512], f32, tag="sc")
                    nc.tensor.matmul(
                        s_ps[:, 0:wc], lhs_q,
                        kT_sb[half:half + 64, c * 512:c * 512 + wc],
                        start=True, stop=True)
                    if c == i // 4:
                        off = i * 128 - c * 512
                        nc.vector.tensor_add(s_ps[:, off:off + 128],
                                             s_ps[:, off:off + 128],
                                             md_sb[:, tsl])
                    nc.scalar.activation(P[:, c * 512:c * 512 + wc],
                                         s_ps[:, 0:wc], EXP,
                                         accum_out=sums4[:, c:c + 1])
                if nch > 1:
                    sums = stat.tile([128, 1], f32, tag="sums")
                    nc.vector.reduce_sum(sums[:], sums4[:, 0:nch], axis=AX)
                else:
                    sums = sums4
                nc.vector.reciprocal(recip[:, j:j + 1], sums[:, 0:1])

                pend.append((j, P))
                if len(pend) > 2:
                    _pv_phase(*pend.pop(0))
            while pend:
                _pv_phase(*pend.pop(0))

            # ---- normalize + transpose o (DMA xbar) ----
            o_sb = work.tile([128, MQ], bf16, tag="o_sb")
            nc.vector.tensor_tensor(
                o_sb[:].rearrange("p (h d) -> p h d", d=HD),
                o_ps[:].rearrange("p (h d) -> p h d", d=HD),
                recip[:].unsqueeze(-1).broadcast_to([128, QH, HD]),
                op=MULT)
            for mt in range(4):
                nc.sync.dma_start_transpose(oT_sb[mt][:, tsl],
                                            o_sb[:, mt * 128:(mt + 1) * 128])

        _emit_wo(NT - 1)

    return nc


# ---------------------------------------------------------------------------
# Host-side input prep / gather
def _rope_tables(w, with_scale):
    j = np.arange(0, HD, 2, dtype=np.float64)
    inv_freq = THETA ** (-j / HD)
    t = np.arange(T, dtype=np.float64)
    f = t[:, None] * inv_freq[None, :]
    c, s = np.cos(f), np.sin(f)
    we, wo = w[0::2].astype(np.float64), w[1::2].astype(np.float64)
    sc = SCALE if with_scale else 1.0
    A = (c * we * sc).astype(np.float32)
    Bt = (s * wo * sc).astype(np.float32)
    C = (s * we * sc).astype(np.float32)
    Dt = (c * wo * sc).astype(np.float32)
    packed = np.zeros((128, NT * 128), dtype=np.float32)
    for i in range(NT):
        rows = slice(i * 128, (i + 1) * 128)
        packed[:, i * 128 + 0:i * 128 + 32] = A[rows]
        packed[:, i * 128 + 32:i * 128 + 64] = Bt[rows]
        packed[:, i * 128 + 64:i * 128 + 96] = C[rows]
        packed[:, i * 128 + 96:i * 128 + 128] = Dt[rows]
    return packed


_CACHE = {}


def _prep(x, Wq, Wk, Wv, Wo, q_norm_w, k_norm_w, mask):
    x = np.asarray(x, dtype=np.float32)
    Wq = np.asarray(Wq, dtype=np.float32)
    Wk = np.asarray(Wk, dtype=np.float32)
    Wv = np.asarray(Wv, dtype=np.float32)
    Wo = np.asarray(Wo, dtype=np.float32)
    mask2d = np.asarray(mask, dtype=np.float32).reshape(T, T)

    ropeq = _rope_tables(np.asarray(q_norm_w, np.float32), True)
    ropek = _rope_tables(np.asarray(k_norm_w, np.float32), False)
    madd = np.zeros((128, NT * 128), dtype=np.float32)
    for i in range(NT):
        blk = mask2d[i * 128:(i + 1) * 128, i * 128:(i + 1) * 128]
        madd[:, i * 128:(i + 1) * 128] = np.maximum(blk, -1e9)

    xT = [np.ascontiguousarray(x[b].T).astype(BF16) for b in range(B)]
    in_maps = []
    for c in range(N_CORES):
        b, g = c // 4, c % 4
        Wq_g = Wq[g * MQ:(g + 1) * MQ]
        Wo_g = Wo[:, g * MQ:(g + 1) * MQ]
        # permute head blocks: position j holds head PERM[j]
        Wq_gp = np.concatenate([Wq_g[h * HD:(h + 1) * HD] for h in PERM], axis=0)
        Wo_gp = np.concatenate([Wo_g[:, h * HD:(h + 1) * HD] for h in PERM], axis=1)
        kvrows = slice(g * MKV, (g + 1) * MKV)
        wqkvT = np.concatenate(
            [Wq_gp.T, Wk[kvrows].T, Wv[kvrows].T], axis=1).astype(BF16)
        woT = np.ascontiguousarray(Wo_gp.T).astype(BF16)
        in_maps.append(dict(
            xT=xT[b], wqkvT=np.ascontiguousarray(wqkvT), woT=woT,
            ropeq=ropeq, ropek=ropek, madd=madd,
            ident=np.eye(128, dtype=BF16)))
    return in_maps


def _run(inputs, trace=False):
    _install_axon_hooks_shim()
    _install_wait_split_patch()
    from concourse.bass_utils import run_bass_kernel_spmd

    if "nc" not in _CACHE:
        _CACHE["nc"] = _build_nc()
    nc = _CACHE["nc"]
    in_maps = _prep(**inputs)
    res = run_bass_kernel_spmd(nc, in_maps, list(range(N_CORES)), trace=trace)
    y = np.zeros((B, T, D), dtype=np.float32)
    for c in range(N_CORES):
        y[c // 4] += res.results[c]["out"].astype(np.float32)
    return y, res


def kernel(x, Wq, Wk, Wv, Wo, q_norm_w, k_norm_w, mask, **_unused):
    y, _ = _run(dict(x=x, Wq=Wq, Wk=Wk, Wv=Wv, Wo=Wo, q_norm_w=q_norm_w,
                     k_norm_w=k_norm_w, mask=mask))
    return y
